# revision 88
# baseline (speedup 1.0000x reference)
"""BeitSelfAttention block-sparse attention kernel for 8 Trainium2 NeuronCores.

Strategy (data-parallel over batch, B=8 -> one batch element per core):
  - Host pre-transposes hidden states (hsT [768,1569] bf16 per core) and
    pre-gathers the relative-position bias as exp(bias)*multiplicity tables
    (index math only; all heavy FLOPs stay on device).
  - Device per core: QKV projections on PE (bf16, fp32 psum accumulate).
    qT gets bias+scale folded into the PSUM->SBUF copy (ACT activation with
    per-partition bias), kT is a plain copy, V keeps a ones-rider bias matmul
    and is stored token-major in pair tiles (cls token merged into pair 0's
    projection).
  - Block-sparse attention computed TRANSPOSED: per key-block-pair p (98 keys)
    and head h, scores simT = kT_pair^T @ qT[:, attending-query-cols] go to
    PSUM; softmax = exp on ACT * exp(bias) on DVE; AV uses V_pair as the
    stationary operand with a ones-column rider accumulating the softmax
    denominator into outT[65, cols] per (head, 512-col quarter) in PSUM.
  - The emission is software-pipelined in two regions: region 1 emits all
    projections plus the first CAP (head, group) score/exp/mult units (so ACT
    overlaps the PE-bound projection phase); region 2 interleaves the
    remaining units with the AV matmuls, quarter tails (cls-key AV close +
    PSUM->SBUF copy), and a per-head normalization chain
    (DVE reciprocal -> SBUF broadcast DMA -> bf16 DVE multiply -> one DMA).
  - Host reassembles [8, 1569, 768].
"""

import os
from collections import deque
from contextlib import ExitStack

import numpy as np

NCLS, BS, NBLK, NPAIR, NH, DH = 1, 49, 32, 16, 12, 64
B, S, D = 8, 1569, 768
NTOK = S - NCLS  # 1568
SCALE = 0.125
N_CORES = 8
SPAD = 1632  # kT/hsT padded width so 128-col stationary slices stay in bounds
CAP = 39     # (head, group) units emitted during the projection region


# ----------------------------------------------------------------------------
# host-side layout
# ----------------------------------------------------------------------------

def _build_layout(rand_idx):
    rand_idx = np.asarray(rand_idx)
    mult = np.zeros((NBLK, NBLK), np.int32)
    for m in range(NBLK):
        for o in (-1, 0, 1):
            mult[m, (m + o) % NBLK] += 1
        for r in rand_idx[m]:
            mult[m, int(r)] += 1

    segs = []
    gcol = 0  # global packed column across banks
    for p in range(NPAIR):
        att = sorted(set(np.nonzero(mult[:, 2 * p])[0]) | set(np.nonzero(mult[:, 2 * p + 1])[0]))
        cols = {0}
        for m in att:
            cols.update(range(1 + BS * m, 1 + BS * (m + 1)))
        cols = sorted(cols)
        runs = []
        c0 = cols[0]
        prev = cols[0]
        for c in cols[1:]:
            if c != prev + 1:
                runs.append((c0, prev - c0 + 1))
                c0 = c
            prev = c
        runs.append((c0, prev - c0 + 1))
        cur = None
        for (rc, rw) in runs:
            while rw > 0:
                take = min(rw, 512 - (gcol % 512))
                if cur is None or cur["bank"] != gcol // 512:
                    cur = {"p": p, "runs": [], "width": 0,
                           "bank": gcol // 512, "off": gcol % 512}
                    segs.append(cur)
                cur["runs"].append((rc, take))
                cur["width"] += take
                gcol += take
                rc += take
                rw -= take
                if gcol % 512 == 0:
                    cur = None
        cur = None  # next pair starts a new segment

    nbank = (gcol + 511) // 512
    ng = (nbank + 1) // 2
    for sg in segs:
        sg["acol"] = sg["bank"] * 512 + sg["off"]
        sg["g"] = sg["bank"] // 2
        sg["goff"] = (sg["bank"] % 2) * 512 + sg["off"]

    segs.sort(key=lambda s: (s["g"], s["bank"], s["off"]))
    groups = [[] for _ in range(ng)]
    for sg in segs:
        groups[sg["g"]].append(sg)

    # AV runs: outT lives as four per-bank quarter tiles [65, <=512].  Split
    # score runs at 512-col quarter boundaries AND at already-written/fresh
    # column transitions (PSUM has_written semantics); tag the first matmul
    # per quarter.
    touched = [False] * 4
    written = np.zeros(S, bool)
    for sg in segs:
        av = []
        oc = 0
        for (rc, rw) in sg["runs"]:
            c, w = rc, rw
            while w > 0:
                bnd = ((c // 512) + 1) * 512
                take = min(w, bnd - c)
                sub0 = c
                while sub0 < c + take:
                    st = bool(written[sub0])
                    sub1 = sub0
                    while sub1 < c + take and bool(written[sub1]) == st:
                        sub1 += 1
                    bnk = sub0 // 512
                    av.append({"qc0": sub0, "w": sub1 - sub0,
                               "oc": oc + (sub0 - c), "first": not touched[bnk]})
                    touched[bnk] = True
                    sub0 = sub1
                written[c:c + take] = True
                oc += take
                c += take
                w -= take
        sg["av_runs"] = av

    gocc = [max(0, min(1024, gcol - g * 1024)) for g in range(ng)]
    last_touch = [0] * 4
    for sg in segs:
        for av in sg["av_runs"]:
            last_touch[av["qc0"] // 512] = max(last_touch[av["qc0"] // 512], sg["g"])
    return {"segs": segs, "groups": groups, "mult": mult, "NBANK": nbank,
            "NG": ng, "last_touch": last_touch, "gocc": gocc}


def _build_ebias(lay, rel_table, rel_pos_index):
    mult = lay["mult"]
    ng = lay["NG"]
    eb = np.zeros((NH, 98, ng * 1024), np.float32)
    for sg in lay["segs"]:
        p = sg["p"]
        ktok = 1 + 98 * p + np.arange(98)
        kblk = 2 * p + np.arange(98) // BS
        acol = sg["acol"]
        for (rc, rw) in sg["runs"]:
            qtok = np.arange(rc, rc + rw)
            qblk = np.maximum(qtok - 1, 0) // BS
            m = mult[qblk][:, kblk].T.astype(np.float32)  # [98, rw]
            m[:, qtok == 0] = 1.0
            idx = rel_pos_index[qtok[:, None], ktok[None, :]]  # [rw, 98]
            val = rel_table[idx]  # [rw, 98, NH]
            ebv = np.exp(val.astype(np.float32)) * m.T[:, :, None]
            eb[:, :, acol:acol + rw] = ebv.transpose(2, 1, 0)
            acol += rw
    return eb


def _build_ebias_cls(rel_table, rel_pos_index):
    idx = rel_pos_index[np.arange(S), 0]
    return np.exp(rel_table[idx].astype(np.float32)).T.copy()  # [NH, S]


# ----------------------------------------------------------------------------
# walrus workaround: split the TileContext tail drain's sem waits
# ----------------------------------------------------------------------------

def _patch_tile_drain():
    import concourse.tile as tile
    from concourse.vector_clock import ScopedClock, VectorClock

    if getattr(tile.TileContext, "_beit_drain_patch", False):
        return

    def _drain_and_barrier(self, tick_clock, wait_clock):
        gc_vec = tick_clock.global_clock
        n = len(gc_vec)
        nonzero = [i for i in range(n) if gc_vec[i] > 0] or [0]
        for i in range(0, len(nonzero), 1):
            chunk = set(nonzero[i:i + 1])
            vec = VectorClock([gc_vec[j] if j in chunk else 0 for j in range(n)])
            drain_inst = self.nc.sync.drain()
            wait_clock.add_sem_waits(drain_inst.ins, ScopedClock({None: vec}))
        self.nc.all_engine_barrier()
        assert self.sems is not None
        popped = self.nc._tile_sem_poison_stack.pop()
        assert popped is self._sem_poison
        self.nc.clear_and_free_semaphores(list(self.sems.allocated().values()))
        self.nc.all_engine_barrier()

    tile.TileContext._drain_and_barrier = _drain_and_barrier
    tile.TileContext._beit_drain_patch = True


def _split_excess_waits(nc, mybir, limit=1):
    """This walrus build allows very few sem waits per instruction; move the
    excess onto EventSemaphore carrier instructions inserted just before."""
    ctr = [0]
    for f in nc.m.functions:
        for bb in f.blocks:
            il = bb.instructions
            out = []
            for inst in il:
                si = inst.sync_info
                if si is not None and si.on_wait and len(si.on_wait) > limit:
                    waits = list(si.on_wait)
                    over = waits[limit:]
                    for j in range(0, len(over), limit):
                        ctr[0] += 1
                        ev = mybir.InstEventSemaphore(
                            name=f"WSPLIT-{ctr[0]}", ins=[], outs=[],
                            engine=inst.engine,
                            sync_info=mybir.SyncInfo(on_wait=over[j:j + limit],
                                                     on_update=[]),
                        )
                        nc.register_instruction(ev, overwrite=True)
                        out.append(ev)
                    si.on_wait = waits[:limit]
                out.append(inst)
            il[:] = out
    return ctr[0]


# ----------------------------------------------------------------------------
# device kernel emission
# ----------------------------------------------------------------------------

def _emit(nc, tile, mybir, lay):
    import concourse.bass as bass

    bf = mybir.dt.bfloat16
    f32 = mybir.dt.float32
    ng = lay["NG"]

    hsT_d = nc.dram_tensor("hsT", [D, S], bf, kind="ExternalInput")
    wq_d = nc.dram_tensor("Wq", [D, D], bf, kind="ExternalInput")
    wk_d = nc.dram_tensor("Wk", [D, D], bf, kind="ExternalInput")
    wv_d = nc.dram_tensor("Wv", [D, D], bf, kind="ExternalInput")
    bqp_d = nc.dram_tensor("bqp", [128, 6], f32, kind="ExternalInput")
    bv_d = nc.dram_tensor("bv_row", [1, D], bf, kind="ExternalInput")
    eb_d = nc.dram_tensor("ebias", [NH, 98, ng * 1024], bf, kind="ExternalInput")
    ebc_d = nc.dram_tensor("ebias_cls", [NH, S], bf, kind="ExternalInput")
    bdo_d = nc.dram_tensor("bd_ones", [NH, NH * 65 + 64], bf, kind="ExternalInput")
    out_d = nc.dram_tensor("out_t", [NH, DH, S], bf, kind="ExternalOutput")

    Exp = mybir.ActivationFunctionType.Exp
    Ident = mybir.ActivationFunctionType.Identity
    s_chunks = [(0, 512), (512, 512), (1024, 512), (1536, S - 1536)]
    quarters = [(0, 512), (512, 512), (1024, 512), (1536, S - 1536)]

    with tile.TileContext(nc) as tc, ExitStack() as ctx:
        consts = ctx.enter_context(tc.tile_pool(name="consts", bufs=1))
        persist = ctx.enter_context(tc.tile_pool(name="persist", bufs=1))

        ones_row = consts.tile([1, 2], bf, tag="ones", name="ones")
        nc.vector.memset(ones_row[:, :], 1.0)
        bqp_sb = consts.tile([128, 6], f32, tag="bqp", name="bqp")
        nc.sync.dma_start(out=bqp_sb[:, :], in_=bqp_d[:, :])
        bv_sb = consts.tile([1, D], bf, tag="bv", name="bv")
        nc.sync.dma_start(out=bv_sb[:, :], in_=bv_d[:, :])
        bv_bc = consts.tile([98, D], bf, tag="bvbc", name="bvbc")
        nc.sync.dma_start(out=bv_bc[:, :],
                          in_=bass.AP(tensor=bv_d, offset=0,
                                      ap=[[0, 98], [1, D]]))

        qT = [persist.tile([128, S], bf, tag=f"qT{t}", name=f"qT{t}") for t in range(6)]
        kT = [persist.tile([128, SPAD], bf, tag=f"kT{t}", name=f"kT{t}") for t in range(6)]
        for t in range(6):
            nc.vector.memset(kT[t][:, S:SPAD], 0.0)
        vst = persist.tile([98, NPAIR * NH * 65 + 64], bf, tag="vst", name="vst")
        nc.vector.memset(vst[:, NPAIR * NH * 65:], 0.0)
        bdv = persist.tile([NH, NH * 65 + 64], bf, tag="bdv", name="bdv")
        bdk = persist.tile([128, 6, NH], bf, tag="bdk", name="bdk")
        atc = persist.tile([NH, S], bf, tag="aTcls", name="aTcls")
        ebc_sb = persist.tile([NH, S], bf, tag="ebc", name="ebc")
        nc.sync.dma_start(out=ebc_sb[:, :], in_=ebc_d[:, :])
        nc.sync.dma_start(out=bdv[:, :], in_=bdo_d[:, :])
        nc.vector.memset(bdk[:, :, :], 0.0)

        # pools that span both emission regions
        scps = ctx.enter_context(tc.tile_pool(name="scps", bufs=2, space="PSUM"))
        arp = ctx.enter_context(tc.tile_pool(name="arp", bufs=3))
        atp = ctx.enter_context(tc.tile_pool(name="atp", bufs=CAP - 4))
        atsp = ctx.enter_context(tc.tile_pool(name="atsp", bufs=6))
        ebpa = ctx.enter_context(tc.tile_pool(name="ebpa", bufs=1))
        ebpb = ctx.enter_context(tc.tile_pool(name="ebpb", bufs=1))

        units = [(h, g) for h in range(NH) for g in range(ng)]
        pending = deque()       # (h, g, aT) with aT ready, AV not yet emitted
        ebh_by_h = {}
        ob_by_h = {}
        outT_by_h = {}
        tails_left = {h: 4 for h in range(NH)}

        def emit_unit(h, g):
            dt = h // 2
            r0 = (h % 2) * 64
            sc = scps.tile([128, 1024], f32, tag="sc", name="sc")
            for sg in lay["groups"][g]:
                kc0 = 1 + 98 * sg["p"]
                oc = 0
                for (rc, rw) in sg["runs"]:
                    nc.tensor.matmul(
                        sc[:, sg["goff"] + oc:sg["goff"] + oc + rw],
                        lhsT=kT[dt][r0:r0 + 64, kc0:kc0 + 128],
                        rhs=qT[dt][r0:r0 + 64, rc:rc + rw],
                        start=True, stop=True,
                    )
                    oc += rw
            gw = lay["gocc"][g]
            half = (ng * 1024) // 2
            if h not in ebh_by_h:
                eba = ebpa.tile([98, half], bf, tag="ebA", name="ebA")
                nc.gpsimd.dma_start(out=eba[:, :], in_=eb_d[h, :, 0:half])
                ebh_by_h[h] = [eba, None]
            if g * 1024 >= half and ebh_by_h[h][1] is None:
                ebb = ebpb.tile([98, half], bf, tag="ebB", name="ebB")
                nc.gpsimd.dma_start(out=ebb[:, :], in_=eb_d[h, :, half:])
                ebh_by_h[h][1] = ebb
            ar = arp.tile([98, 1024], bf, tag="ar", name="ar")
            nc.scalar.activation(ar[:, :gw], sc[0:98, :gw], Exp)
            if gw > 64:
                aT = atp.tile([98, 1024], bf, tag="aT", name="aT")
            else:
                aT = atsp.tile([98, 64], bf, tag="aTs", name="aTs")
            ebt = ebh_by_h[h][0] if g * 1024 < half else ebh_by_h[h][1]
            ebo = g * 1024 - (0 if g * 1024 < half else half)
            nc.vector.tensor_mul(aT[:, :gw], ar[:, :gw], ebt[:, ebo:ebo + gw])
            pending.append((h, g, aT))

        def emit_head_tail(h, q, outT):
            # cls-key AV (K=12 block-diag v_cls); closes this quarter's psum
            # accumulation.  Copy [65, qw] (rows 0..63 out, row 64 denom) into
            # the head's staging tile as bf16.
            qb, qw = quarters[q]
            nc.tensor.matmul(
                outT[:, 0:qw],
                lhsT=bdv[:, h * 65:h * 65 + 128],
                rhs=atc[:, qb:qb + qw],
                start=False, stop=True,
            )
            ob = ob_by_h[h]
            if q % 2 == 0:
                nc.scalar.activation(ob[:, qb:qb + qw], outT[0:65, :qw],
                                     mybir.ActivationFunctionType.Copy)
            else:
                nc.vector.tensor_copy(ob[:, qb:qb + qw], outT[0:65, :qw])
            tails_left[h] -= 1
            if tails_left[h] == 0:
                emit_norm(h)

        def emit_norm(h):
            # the last heads' chains sit on the drain critical path: split
            # them in column halves so recip/broadcast/mult/out pipeline
            ob = ob_by_h[h]
            den_r = nrm.tile([1, S], bf, tag="denr", name="denr")
            den_dr = drp.tile([1, S], bf, tag="dend", name="dend")
            bc = nrm.tile([64, S], bf, tag="bc", name="bc")
            obm = nrm.tile([64, S], bf, tag="obm", name="obm")
            halves = ((0, 768), (768, S)) if h >= 10 else ((0, S),)
            for (c0, c1) in halves:
                with nc.allow_low_precision("bf16 softmax denominator reciprocal"):
                    nc.vector.reciprocal(den_r[0:1, c0:c1], ob[64:65, c0:c1])
                nc.sync.dma_start(out=den_dr[:, c0:c1], in_=den_r[0:1, c0:c1])
                src = den_dr[:, c0:c1]
                bcast = bass.AP(tensor=src.tensor, offset=src.offset,
                                ap=[[0, 64]] + [list(d) for d in src.ap][1:])
                nc.sync.dma_start(out=bc[:, c0:c1], in_=bcast)
                nc.vector.tensor_mul(obm[:, c0:c1], ob[0:64, c0:c1], bc[:, c0:c1])
                nc.sync.dma_start(out=out_d[h][:, c0:c1], in_=obm[:, c0:c1])

        def emit_av(h, g, aT):
            if h not in outT_by_h:
                outT_by_h[h] = [
                    otps.tile([128, qw], f32, tag=f"outQ{q}", name=f"outQ{q}")
                    for q, (qb, qw) in enumerate(quarters)
                ]
                ob_by_h[h] = obp.tile([65, S], bf, tag="ob", name="ob")
            outTs = outT_by_h[h]
            for sg in lay["groups"][g]:
                vh = vst[0:98, sg["p"] * NH * 65 + h * 65:sg["p"] * NH * 65 + h * 65 + 128]
                for av in sg["av_runs"]:
                    q = av["qc0"] // 512
                    lc = av["qc0"] - 512 * q
                    nc.tensor.matmul(
                        outTs[q][:, lc:lc + av["w"]],
                        lhsT=vh,
                        rhs=aT[0:98, sg["goff"] + av["oc"]:sg["goff"] + av["oc"] + av["w"]],
                        start=av["first"], stop=False,
                    )
            for q in range(4):
                if lay["last_touch"][q] == g:
                    emit_head_tail(h, q, outTs[q])
            if g == ng - 1:
                outT_by_h.pop(h)

        # ---------------- region 1: projections + first CAP units ----------
        n_emitted = 0
        with tc.tile_pool(name="phA", bufs=1) as phA, \
             tc.tile_pool(name="wst", bufs=2) as wst, \
             tc.tile_pool(name="pp", bufs=4, space="PSUM") as pp:
            def load_w(dram):
                tiles = []
                for t in range(6):
                    wt = wst.tile([128, D], bf, tag=f"w{t}", name=f"w{t}")
                    nc.sync.dma_start(out=wt[:, :], in_=dram[t * 128:(t + 1) * 128, :])
                    tiles.append(wt)
                return tiles

            wk = load_w(wk_d)
            hsT = []
            for t in range(6):
                hst = phA.tile([128, SPAD], bf, tag=f"hsT{t}", name=f"hsT{t}")
                nc.sync.dma_start(out=hst[:, 0:512], in_=hsT_d[t * 128:(t + 1) * 128, 0:512])
                nc.vector.memset(hst[:, S:SPAD], 0.0)
                hsT.append(hst)
            for (h0, h1) in ((512, 1024), (1024, S)):
                for t in range(6):
                    nc.sync.dma_start(out=hsT[t][:, h0:h1], in_=hsT_d[t * 128:(t + 1) * 128, h0:h1])

            vst4 = vst[:, 0:NPAIR * NH * 65].rearrange("a (p h e) -> a p h e", p=NPAIR, h=NH)
            nc.vector.memset(vst4[:, :, :, 64:65], 1.0)
            vcls_sb = consts.tile([1, D], bf, tag="vcls", name="vcls")
            wv = [None]

            def emit_v_pair(p):
                # token-major V projection for one 98-token pair; psum chunks
                # reuse the "pq" tag so V interleaves with the q/k projection
                c0 = 1 + 98 * p
                for ci, (h0, hw) in enumerate(((0, 512), (512, 256))):
                    ps = pp.tile([128, 512], f32, tag="pq", name="pq")
                    for kt in range(6):
                        nc.tensor.matmul(
                            ps[:, :hw],
                            lhsT=hsT[kt][:, c0:c0 + 128],
                            rhs=wv[0][kt][:, h0:h0 + hw],
                            start=(kt == 0), stop=(kt == 5),
                        )
                    nh0 = h0 // 64
                    dst = vst4[:, p, nh0:nh0 + hw // 64, 0:64]
                    src = ps[0:98, :hw].rearrange("a (h e) -> a h e", e=64)
                    bvs = bv_bc[0:98, h0:h0 + hw].rearrange("a (h e) -> a h e", e=64)
                    nc.vector.tensor_add(dst, src, bvs)

            def emit_v_cls():
                # cls-token V row: [1, 768] out, +bv folded into the copy
                for (h0, hw) in ((0, 512), (512, 256)):
                    ps = pp.tile([128, 512], f32, tag="pq", name="pq")
                    for kt in range(6):
                        nc.tensor.matmul(
                            ps[0:1, :hw],
                            lhsT=hsT[kt][:, 0:1],
                            rhs=wv[0][kt][:, h0:h0 + hw],
                            start=(kt == 0), stop=(kt == 5),
                        )
                    nc.vector.tensor_add(vcls_sb[:, h0:h0 + hw], ps[0:1, :hw],
                                         bv_sb[0:1, h0:h0 + hw])

            # kT projection first (cls-score row needs the full kT col 0)
            for dt in range(6):
                for (c0, cw) in s_chunks:
                    ps = pp.tile([128, 512], f32, tag="pq", name="pq")
                    for kt in range(6):
                        nc.tensor.matmul(
                            ps[:, :cw],
                            lhsT=wk[kt][:, dt * 128:(dt + 1) * 128],
                            rhs=hsT[kt][:, c0:c0 + cw],
                            start=(kt == 0), stop=(kt == 5),
                        )
                    nc.scalar.copy(kT[dt][:, c0:c0 + cw], ps[:, :cw])
                # block-diag cls-key columns for the dense cls-key score row
                for half in range(2):
                    r0 = half * 64
                    nc.vector.tensor_copy(
                        bdk[r0:r0 + 64, dt, 2 * dt + half:2 * dt + half + 1],
                        kT[dt][r0:r0 + 64, 0:1],
                    )

            # qT projection (bias+scale folded into the PSUM->SBUF copy),
            # interleaved with per-head-pair score units and V pairs
            wq = load_w(wq_d)
            wv[0] = load_w(wv_d)
            vp = 0
            clsraw = phA.tile([NH, S], bf, tag="clsraw", name="clsraw")
            for dt in range(6):
                for (c0, cw) in s_chunks:
                    ps = pp.tile([128, 512], f32, tag="pq", name="pq")
                    for kt in range(6):
                        nc.tensor.matmul(
                            ps[:, :cw],
                            lhsT=wq[kt][:, dt * 128:(dt + 1) * 128],
                            rhs=hsT[kt][:, c0:c0 + cw],
                            start=(kt == 0), stop=(kt == 5),
                        )
                    nc.scalar.activation(qT[dt][:, c0:c0 + cw], ps[:, :cw], Ident,
                                         bias=bqp_sb[:, dt:dt + 1], scale=SCALE)
                for h in (2 * dt, 2 * dt + 1):
                    for g in range(ng):
                        if n_emitted < CAP:
                            emit_unit(h, g)
                            n_emitted += 1
                while vp < min(NPAIR, 2 + (dt + 1) * 2):
                    emit_v_pair(vp)
                    vp += 1
                    if vp == 1:
                        emit_v_cls()
                if dt == 5:
                    # dense cls-key score row: [12, S] in 512-col psum chunks
                    # (borrows the score-tile psum pool)
                    for (c0, cw) in s_chunks:
                        cpt = scps.tile([128, 1024], f32, tag="sc", name="sc")
                        cp = cpt[0:12, 0:512]
                        for t in range(6):
                            nc.tensor.matmul(
                                cp[:, :cw],
                                lhsT=bdk[:, t, :],
                                rhs=qT[t][:, c0:c0 + cw],
                                start=(t == 0), stop=(t == 5),
                            )
                        nc.scalar.activation(clsraw[:, c0:c0 + cw], cp[:, :cw], Exp)
                        nc.vector.tensor_mul(atc[:, c0:c0 + cw], clsraw[:, c0:c0 + cw],
                                             ebc_sb[:, c0:c0 + cw])
            while vp < NPAIR:
                emit_v_pair(vp)
                vp += 1
            for h in range(NH):
                nc.sync.dma_start(
                    out=bdv[h:h + 1, h * 65:h * 65 + 64],
                    in_=vcls_sb[0:1, h * 64:(h + 1) * 64],
                )

        # ---------------- region 2: remaining units + AV pipeline ----------
        with tc.tile_pool(name="otps", bufs=1, space="PSUM") as otps_, \
             tc.tile_pool(name="obp", bufs=2) as obp_, \
             tc.tile_pool(name="nrm", bufs=2) as nrm_, \
             tc.tile_pool(name="drp", bufs=2, space="DRAM") as drp_:
            otps, obp, nrm, drp = otps_, obp_, nrm_, drp_
            rem = units[n_emitted:]
            for i, (h, g) in enumerate(rem):
                if pending:
                    emit_av(*pending.popleft())
                if pending and (i < 32 or len(rem) - i <= len(pending)):
                    emit_av(*pending.popleft())
                emit_unit(h, g)
            while pending:
                emit_av(*pending.popleft())

    _split_excess_waits(nc, mybir, limit=1)
    return nc


def _bench_pjrt(nc, in_maps, n_cores, iters=20, warmup=3):
    """Time repeated executions of the compiled kernel (no donation; inputs
    stay device-resident).  Returns (per_iter_ns, results_list)."""
    import time

    import jax
    import numpy as np
    from jax.sharding import Mesh, PartitionSpec
    from jax.experimental.shard_map import shard_map

    from concourse import mybir
    from concourse.bass2jax import (_bass_exec_p, install_neuronx_cc_hook,
                                    partition_id_tensor)

    install_neuronx_cc_hook()
    partition_name = nc.partition_id_tensor.name if nc.partition_id_tensor else None
    in_names, out_names, out_avals, zero_outs = [], [], [], []
    for alloc in nc.m.functions[0].allocations:
        if not isinstance(alloc, mybir.MemoryLocationSet):
            continue
        name = alloc.memorylocations[0].name
        if alloc.kind == "ExternalInput":
            if name != partition_name:
                in_names.append(name)
        elif alloc.kind == "ExternalOutput":
            shape = tuple(alloc.tensor_shape)
            dtype = mybir.dt.np(alloc.dtype)
            out_names.append(name)
            out_avals.append(jax.core.ShapedArray(shape, dtype))
            zero_outs.append(np.zeros(shape, dtype))
    n_params = len(in_names)
    all_in_names = in_names + out_names + ([partition_name] if partition_name else [])

    def _body(*args):
        operands = list(args)
        if partition_name is not None:
            operands.append(partition_id_tensor())
        return tuple(_bass_exec_p.bind(
            *operands,
            out_avals=tuple(out_avals),
            in_names=tuple(all_in_names),
            out_names=tuple(out_names),
            lowering_input_output_aliases=(),
            sim_require_finite=True,
            sim_require_nnan=True,
            nc=nc,
        ))

    devices = jax.devices()[:n_cores]
    mesh = Mesh(np.asarray(devices), ("core",))
    n_outs = len(out_names)
    sharded = jax.jit(
        shard_map(_body, mesh=mesh,
                  in_specs=(PartitionSpec("core"),) * (n_params + n_outs),
                  out_specs=(PartitionSpec("core"),) * n_outs,
                  check_rep=False),
        keep_unused=True,
    )
    per_core = [[np.asarray(m[name]) for name in in_names] for m in in_maps]
    concat_in = [np.concatenate([per_core[c][i] for c in range(n_cores)], axis=0)
                 for i in range(n_params)]
    concat_zeros = [np.zeros((n_cores * z.shape[0], *z.shape[1:]), z.dtype)
                    for z in zero_outs]
    dev_in = [jax.device_put(a) for a in concat_in + concat_zeros]
    out = sharded(*dev_in)
    jax.block_until_ready(out)
    for _ in range(warmup):
        out = sharded(*dev_in)
    jax.block_until_ready(out)
    t0 = time.perf_counter()
    for _ in range(iters):
        out = sharded(*dev_in)
    jax.block_until_ready(out)
    dt = (time.perf_counter() - t0) / iters
    results = [
        {name: np.asarray(out[i]).reshape(n_cores, *out_avals[i].shape)[c]
         for i, name in enumerate(out_names)}
        for c in range(n_cores)
    ]
    return int(dt * 1e9), results


# ----------------------------------------------------------------------------
# public entry point
# ----------------------------------------------------------------------------

def kernel(hidden_states, Wq, bq, Wk, Wv, bv, rel_table, rel_pos_index, rand_idx):
    import ml_dtypes

    import concourse.bass as bass
    import concourse.tile as tile
    from concourse import mybir
    from concourse.bass_utils import run_bass_kernel_spmd

    _patch_tile_drain()
    bf16 = ml_dtypes.bfloat16

    hidden_states = np.asarray(hidden_states, np.float32)
    Wq = np.asarray(Wq, np.float32)
    Wk = np.asarray(Wk, np.float32)
    Wv = np.asarray(Wv, np.float32)
    bq = np.asarray(bq, np.float32)
    bv = np.asarray(bv, np.float32)
    rel_table = np.asarray(rel_table, np.float32)
    rel_pos_index = np.asarray(rel_pos_index)
    rand_idx = np.asarray(rand_idx)

    lay = _build_layout(rand_idx)
    eb = _build_ebias(lay, rel_table, rel_pos_index).astype(bf16)
    ebc = _build_ebias_cls(rel_table, rel_pos_index).astype(bf16)
    bdo = np.zeros((NH, NH * 65 + 64), np.float32)
    for h in range(NH):
        bdo[h, h * 65 + 64] = 1.0
    bdo = bdo.astype(bf16)
    bqp = (bq.reshape(6, 128).T * SCALE).astype(np.float32).copy()

    shared = {
        "Wq": Wq.astype(bf16), "Wk": Wk.astype(bf16), "Wv": Wv.astype(bf16),
        "bqp": bqp,
        "bv_row": bv.reshape(1, D).astype(bf16),
        "ebias": eb, "ebias_cls": ebc, "bd_ones": bdo,
    }
    in_maps = []
    for b in range(B):
        m = dict(shared)
        m["hsT"] = np.ascontiguousarray(hidden_states[b].T).astype(bf16)
        in_maps.append(m)

    nc = bass.Bass()
    _emit(nc, tile, mybir, lay)

    kernel.last_nc = nc
    kernel.last_in_maps = in_maps
    bench_iters = int(os.environ.get("BEIT_BENCH", "0"))
    if bench_iters > 0:
        per_iter_ns, results = _bench_pjrt(nc, in_maps, N_CORES, iters=bench_iters)
        kernel.last_exec_time_ns = per_iter_ns
    else:
        res = run_bass_kernel_spmd(nc, in_maps, core_ids=list(range(N_CORES)))
        results = res.results

    out = np.empty((B, S, NH * DH), np.float32)
    for b in range(B):
        o = results[b]["out_t"].astype(np.float32)  # [NH, DH, S]
        out[b] = o.transpose(2, 0, 1).reshape(S, NH * DH)
    return out


# revision 93
# speedup vs baseline: 1.0010x; 1.0010x over previous
"""BeitSelfAttention block-sparse attention kernel for 8 Trainium2 NeuronCores.

Strategy (data-parallel over batch, B=8 -> one batch element per core):
  - Host pre-transposes hidden states (hsT [768,1569] bf16 per core) and
    pre-gathers the relative-position bias as exp(bias)*multiplicity tables
    (index math only; all heavy FLOPs stay on device).
  - Device per core: QKV projections on PE (bf16, fp32 psum accumulate).
    qT gets bias+scale folded into the PSUM->SBUF copy (ACT activation with
    per-partition bias), kT is a plain copy, V keeps a ones-rider bias matmul
    and is stored token-major in pair tiles (cls token merged into pair 0's
    projection).
  - Block-sparse attention computed TRANSPOSED: per key-block-pair p (98 keys)
    and head h, scores simT = kT_pair^T @ qT[:, attending-query-cols] go to
    PSUM; softmax = exp on ACT * exp(bias) on DVE; AV uses V_pair as the
    stationary operand with a ones-column rider accumulating the softmax
    denominator into outT[65, cols] per (head, 512-col quarter) in PSUM.
  - The emission is software-pipelined in two regions: region 1 emits all
    projections plus the first CAP (head, group) score/exp/mult units (so ACT
    overlaps the PE-bound projection phase); region 2 interleaves the
    remaining units with the AV matmuls, quarter tails (cls-key AV close +
    PSUM->SBUF copy), and a per-head normalization chain
    (DVE reciprocal -> SBUF broadcast DMA -> bf16 DVE multiply -> one DMA).
  - Host reassembles [8, 1569, 768].
"""

import os
from collections import deque
from contextlib import ExitStack

import numpy as np

NCLS, BS, NBLK, NPAIR, NH, DH = 1, 49, 32, 16, 12, 64
B, S, D = 8, 1569, 768
NTOK = S - NCLS  # 1568
SCALE = 0.125
N_CORES = 8
SPAD = 1632  # kT/hsT padded width so 128-col stationary slices stay in bounds
CAP = 39     # (head, group) units emitted during the projection region


# ----------------------------------------------------------------------------
# host-side layout
# ----------------------------------------------------------------------------

def _build_layout(rand_idx):
    rand_idx = np.asarray(rand_idx)
    mult = np.zeros((NBLK, NBLK), np.int32)
    for m in range(NBLK):
        for o in (-1, 0, 1):
            mult[m, (m + o) % NBLK] += 1
        for r in rand_idx[m]:
            mult[m, int(r)] += 1

    segs = []
    gcol = 0  # global packed column across banks
    for p in range(NPAIR):
        att = sorted(set(np.nonzero(mult[:, 2 * p])[0]) | set(np.nonzero(mult[:, 2 * p + 1])[0]))
        cols = {0}
        for m in att:
            cols.update(range(1 + BS * m, 1 + BS * (m + 1)))
        cols = sorted(cols)
        runs = []
        c0 = cols[0]
        prev = cols[0]
        for c in cols[1:]:
            if c != prev + 1:
                runs.append((c0, prev - c0 + 1))
                c0 = c
            prev = c
        runs.append((c0, prev - c0 + 1))
        cur = None
        for (rc, rw) in runs:
            while rw > 0:
                take = min(rw, 512 - (gcol % 512))
                if cur is None or cur["bank"] != gcol // 512:
                    cur = {"p": p, "runs": [], "width": 0,
                           "bank": gcol // 512, "off": gcol % 512}
                    segs.append(cur)
                cur["runs"].append((rc, take))
                cur["width"] += take
                gcol += take
                rc += take
                rw -= take
                if gcol % 512 == 0:
                    cur = None
        cur = None  # next pair starts a new segment

    nbank = (gcol + 511) // 512
    ng = (nbank + 1) // 2
    for sg in segs:
        sg["acol"] = sg["bank"] * 512 + sg["off"]
        sg["g"] = sg["bank"] // 2
        sg["goff"] = (sg["bank"] % 2) * 512 + sg["off"]

    segs.sort(key=lambda s: (s["g"], s["bank"], s["off"]))
    groups = [[] for _ in range(ng)]
    for sg in segs:
        groups[sg["g"]].append(sg)

    # AV runs: outT lives as four per-bank quarter tiles [65, <=512].  Split
    # score runs at 512-col quarter boundaries AND at already-written/fresh
    # column transitions (PSUM has_written semantics); tag the first matmul
    # per quarter.
    touched = [False] * 4
    written = np.zeros(S, bool)
    for sg in segs:
        av = []
        oc = 0
        for (rc, rw) in sg["runs"]:
            c, w = rc, rw
            while w > 0:
                bnd = ((c // 512) + 1) * 512
                take = min(w, bnd - c)
                sub0 = c
                while sub0 < c + take:
                    st = bool(written[sub0])
                    sub1 = sub0
                    while sub1 < c + take and bool(written[sub1]) == st:
                        sub1 += 1
                    bnk = sub0 // 512
                    av.append({"qc0": sub0, "w": sub1 - sub0,
                               "oc": oc + (sub0 - c), "first": not touched[bnk]})
                    touched[bnk] = True
                    sub0 = sub1
                written[c:c + take] = True
                oc += take
                c += take
                w -= take
        sg["av_runs"] = av

    gocc = [max(0, min(1024, gcol - g * 1024)) for g in range(ng)]
    last_touch = [0] * 4
    for sg in segs:
        for av in sg["av_runs"]:
            last_touch[av["qc0"] // 512] = max(last_touch[av["qc0"] // 512], sg["g"])
    return {"segs": segs, "groups": groups, "mult": mult, "NBANK": nbank,
            "NG": ng, "last_touch": last_touch, "gocc": gocc}


def _build_ebias(lay, rel_table, rel_pos_index):
    mult = lay["mult"]
    ng = lay["NG"]
    eb = np.zeros((NH, 98, ng * 1024), np.float32)
    for sg in lay["segs"]:
        p = sg["p"]
        ktok = 1 + 98 * p + np.arange(98)
        kblk = 2 * p + np.arange(98) // BS
        acol = sg["acol"]
        for (rc, rw) in sg["runs"]:
            qtok = np.arange(rc, rc + rw)
            qblk = np.maximum(qtok - 1, 0) // BS
            m = mult[qblk][:, kblk].T.astype(np.float32)  # [98, rw]
            m[:, qtok == 0] = 1.0
            idx = rel_pos_index[qtok[:, None], ktok[None, :]]  # [rw, 98]
            val = rel_table[idx]  # [rw, 98, NH]
            ebv = np.exp(val.astype(np.float32)) * m.T[:, :, None]
            eb[:, :, acol:acol + rw] = ebv.transpose(2, 1, 0)
            acol += rw
    return eb


def _build_ebias_cls(rel_table, rel_pos_index):
    idx = rel_pos_index[np.arange(S), 0]
    return np.exp(rel_table[idx].astype(np.float32)).T.copy()  # [NH, S]


# ----------------------------------------------------------------------------
# walrus workaround: split the TileContext tail drain's sem waits
# ----------------------------------------------------------------------------

def _patch_tile_drain():
    import concourse.tile as tile
    from concourse.vector_clock import ScopedClock, VectorClock

    if getattr(tile.TileContext, "_beit_drain_patch", False):
        return

    def _drain_and_barrier(self, tick_clock, wait_clock):
        gc_vec = tick_clock.global_clock
        n = len(gc_vec)
        nonzero = [i for i in range(n) if gc_vec[i] > 0] or [0]
        for i in range(0, len(nonzero), 1):
            chunk = set(nonzero[i:i + 1])
            vec = VectorClock([gc_vec[j] if j in chunk else 0 for j in range(n)])
            drain_inst = self.nc.sync.drain()
            wait_clock.add_sem_waits(drain_inst.ins, ScopedClock({None: vec}))
        self.nc.all_engine_barrier()
        assert self.sems is not None
        popped = self.nc._tile_sem_poison_stack.pop()
        assert popped is self._sem_poison
        self.nc.clear_and_free_semaphores(list(self.sems.allocated().values()))
        self.nc.all_engine_barrier()

    tile.TileContext._drain_and_barrier = _drain_and_barrier
    tile.TileContext._beit_drain_patch = True


def _split_excess_waits(nc, mybir, limit=1):
    """This walrus build allows very few sem waits per instruction; move the
    excess onto EventSemaphore carrier instructions inserted just before."""
    ctr = [0]
    for f in nc.m.functions:
        for bb in f.blocks:
            il = bb.instructions
            out = []
            for inst in il:
                si = inst.sync_info
                if si is not None and si.on_wait and len(si.on_wait) > limit:
                    waits = list(si.on_wait)
                    over = waits[limit:]
                    for j in range(0, len(over), limit):
                        ctr[0] += 1
                        ev = mybir.InstEventSemaphore(
                            name=f"WSPLIT-{ctr[0]}", ins=[], outs=[],
                            engine=inst.engine,
                            sync_info=mybir.SyncInfo(on_wait=over[j:j + limit],
                                                     on_update=[]),
                        )
                        nc.register_instruction(ev, overwrite=True)
                        out.append(ev)
                    si.on_wait = waits[:limit]
                out.append(inst)
            il[:] = out
    return ctr[0]


# ----------------------------------------------------------------------------
# device kernel emission
# ----------------------------------------------------------------------------

def _emit(nc, tile, mybir, lay):
    import concourse.bass as bass

    bf = mybir.dt.bfloat16
    f32 = mybir.dt.float32
    ng = lay["NG"]

    hsT_d = nc.dram_tensor("hsT", [D, S], bf, kind="ExternalInput")
    wq_d = nc.dram_tensor("Wq", [D, D], bf, kind="ExternalInput")
    wk_d = nc.dram_tensor("Wk", [D, D], bf, kind="ExternalInput")
    wv_d = nc.dram_tensor("Wv", [D, D], bf, kind="ExternalInput")
    bqp_d = nc.dram_tensor("bqp", [128, 6], f32, kind="ExternalInput")
    bv_d = nc.dram_tensor("bv_row", [1, D], bf, kind="ExternalInput")
    eb_d = nc.dram_tensor("ebias", [NH, 98, ng * 1024], bf, kind="ExternalInput")
    ebc_d = nc.dram_tensor("ebias_cls", [NH, S], bf, kind="ExternalInput")
    bdo_d = nc.dram_tensor("bd_ones", [NH, NH * 65 + 64], bf, kind="ExternalInput")
    out_d = nc.dram_tensor("out_t", [NH, DH, S], bf, kind="ExternalOutput")

    Exp = mybir.ActivationFunctionType.Exp
    Ident = mybir.ActivationFunctionType.Identity
    s_chunks = [(0, 512), (512, 512), (1024, 512), (1536, S - 1536)]
    quarters = [(0, 512), (512, 512), (1024, 512), (1536, S - 1536)]

    with tile.TileContext(nc) as tc, ExitStack() as ctx:
        consts = ctx.enter_context(tc.tile_pool(name="consts", bufs=1))
        persist = ctx.enter_context(tc.tile_pool(name="persist", bufs=1))

        ones_row = consts.tile([1, 2], bf, tag="ones", name="ones")
        nc.vector.memset(ones_row[:, :], 1.0)
        bqp_sb = consts.tile([128, 6], f32, tag="bqp", name="bqp")
        nc.sync.dma_start(out=bqp_sb[:, :], in_=bqp_d[:, :])
        bv_sb = consts.tile([1, D], bf, tag="bv", name="bv")
        nc.sync.dma_start(out=bv_sb[:, :], in_=bv_d[:, :])
        bv_bc = consts.tile([98, D], bf, tag="bvbc", name="bvbc")
        nc.sync.dma_start(out=bv_bc[:, :],
                          in_=bass.AP(tensor=bv_d, offset=0,
                                      ap=[[0, 98], [1, D]]))

        qT = [persist.tile([128, S], bf, tag=f"qT{t}", name=f"qT{t}") for t in range(6)]
        kT = [persist.tile([128, SPAD], bf, tag=f"kT{t}", name=f"kT{t}") for t in range(6)]
        for t in range(6):
            nc.vector.memset(kT[t][:, S:SPAD], 0.0)
        vst = persist.tile([98, NPAIR * NH * 65 + 64], bf, tag="vst", name="vst")
        nc.vector.memset(vst[:, NPAIR * NH * 65:], 0.0)
        bdv = persist.tile([NH, NH * 65 + 64], bf, tag="bdv", name="bdv")
        bdk = persist.tile([128, 6, NH], bf, tag="bdk", name="bdk")
        atc = persist.tile([NH, S], bf, tag="aTcls", name="aTcls")
        ebc_sb = persist.tile([NH, S], bf, tag="ebc", name="ebc")
        nc.sync.dma_start(out=ebc_sb[:, :], in_=ebc_d[:, :])
        nc.sync.dma_start(out=bdv[:, :], in_=bdo_d[:, :])
        nc.vector.memset(bdk[:, :, :], 0.0)

        # pools that span both emission regions
        scps = ctx.enter_context(tc.tile_pool(name="scps", bufs=2, space="PSUM"))
        arp = ctx.enter_context(tc.tile_pool(name="arp", bufs=3))
        atp = ctx.enter_context(tc.tile_pool(name="atp", bufs=CAP - 4))
        atsp = ctx.enter_context(tc.tile_pool(name="atsp", bufs=6))
        ebpa = ctx.enter_context(tc.tile_pool(name="ebpa", bufs=1))
        ebpb = ctx.enter_context(tc.tile_pool(name="ebpb", bufs=1))

        units = [(h, g) for h in range(NH) for g in range(ng)]
        pending = deque()       # (h, g, aT) with aT ready, AV not yet emitted
        ebh_by_h = {}
        ob_by_h = {}
        outT_by_h = {}
        tails_left = {h: 4 for h in range(NH)}

        def emit_unit(h, g):
            dt = h // 2
            r0 = (h % 2) * 64
            sc = scps.tile([128, 1024], f32, tag="sc", name="sc")
            for sg in lay["groups"][g]:
                kc0 = 1 + 98 * sg["p"]
                oc = 0
                for (rc, rw) in sg["runs"]:
                    nc.tensor.matmul(
                        sc[:, sg["goff"] + oc:sg["goff"] + oc + rw],
                        lhsT=kT[dt][r0:r0 + 64, kc0:kc0 + 128],
                        rhs=qT[dt][r0:r0 + 64, rc:rc + rw],
                        start=True, stop=True,
                    )
                    oc += rw
            gw = lay["gocc"][g]
            half = (ng * 1024) // 2
            if h not in ebh_by_h:
                eba = ebpa.tile([98, half], bf, tag="ebA", name="ebA")
                nc.gpsimd.dma_start(out=eba[:, :], in_=eb_d[h, :, 0:half])
                ebh_by_h[h] = [eba, None]
            if g * 1024 >= half and ebh_by_h[h][1] is None:
                ebb = ebpb.tile([98, half], bf, tag="ebB", name="ebB")
                nc.gpsimd.dma_start(out=ebb[:, :], in_=eb_d[h, :, half:])
                ebh_by_h[h][1] = ebb
            ar = arp.tile([98, 1024], bf, tag="ar", name="ar")
            nc.scalar.activation(ar[:, :gw], sc[0:98, :gw], Exp)
            if gw > 64:
                aT = atp.tile([98, 1024], bf, tag="aT", name="aT")
            else:
                aT = atsp.tile([98, 64], bf, tag="aTs", name="aTs")
            ebt = ebh_by_h[h][0] if g * 1024 < half else ebh_by_h[h][1]
            ebo = g * 1024 - (0 if g * 1024 < half else half)
            nc.vector.tensor_mul(aT[:, :gw], ar[:, :gw], ebt[:, ebo:ebo + gw])
            pending.append((h, g, aT))

        def emit_head_tail(h, q, outT):
            # cls-key AV (K=12 block-diag v_cls); closes this quarter's psum
            # accumulation.  Copy [65, qw] (rows 0..63 out, row 64 denom) into
            # the head's staging tile as bf16.
            qb, qw = quarters[q]
            nc.tensor.matmul(
                outT[:, 0:qw],
                lhsT=bdv[:, h * 65:h * 65 + 128],
                rhs=atc[:, qb:qb + qw],
                start=False, stop=True,
            )
            ob = ob_by_h[h]
            if q % 2 == 0:
                nc.scalar.activation(ob[:, qb:qb + qw], outT[0:65, :qw],
                                     mybir.ActivationFunctionType.Copy)
            else:
                nc.vector.tensor_copy(ob[:, qb:qb + qw], outT[0:65, :qw])
            tails_left[h] -= 1
            if tails_left[h] == 0:
                emit_norm(h)

        def emit_norm(h):
            # the last heads' chains sit on the drain critical path: split
            # them in column halves so recip/broadcast/mult/out pipeline
            ob = ob_by_h[h]
            den_r = nrm.tile([1, S], bf, tag="denr", name="denr")
            den_dr = drp.tile([1, S], bf, tag="dend", name="dend")
            bc = nrm.tile([64, S], bf, tag="bc", name="bc")
            obm = nrm.tile([64, S], bf, tag="obm", name="obm")
            halves = ((0, 768), (768, S)) if h >= 10 else ((0, S),)
            for (c0, c1) in halves:
                with nc.allow_low_precision("bf16 softmax denominator reciprocal"):
                    nc.vector.reciprocal(den_r[0:1, c0:c1], ob[64:65, c0:c1])
                nc.sync.dma_start(out=den_dr[:, c0:c1], in_=den_r[0:1, c0:c1])
                src = den_dr[:, c0:c1]
                bcast = bass.AP(tensor=src.tensor, offset=src.offset,
                                ap=[[0, 64]] + [list(d) for d in src.ap][1:])
                nc.sync.dma_start(out=bc[:, c0:c1], in_=bcast)
                nc.vector.tensor_mul(obm[:, c0:c1], ob[0:64, c0:c1], bc[:, c0:c1])
                nc.sync.dma_start(out=out_d[h][:, c0:c1], in_=obm[:, c0:c1])

        def emit_av(h, g, aT):
            if h not in outT_by_h:
                outT_by_h[h] = [
                    otps.tile([128, qw], f32, tag=f"outQ{q}", name=f"outQ{q}")
                    for q, (qb, qw) in enumerate(quarters)
                ]
                ob_by_h[h] = obp.tile([65, S], bf, tag="ob", name="ob")
            outTs = outT_by_h[h]
            for sg in lay["groups"][g]:
                vh = vst[0:98, sg["p"] * NH * 65 + h * 65:sg["p"] * NH * 65 + h * 65 + 128]
                for av in sg["av_runs"]:
                    q = av["qc0"] // 512
                    lc = av["qc0"] - 512 * q
                    nc.tensor.matmul(
                        outTs[q][:, lc:lc + av["w"]],
                        lhsT=vh,
                        rhs=aT[0:98, sg["goff"] + av["oc"]:sg["goff"] + av["oc"] + av["w"]],
                        start=av["first"], stop=False,
                    )
            for q in range(4):
                if lay["last_touch"][q] == g:
                    emit_head_tail(h, q, outTs[q])
            if g == ng - 1:
                outT_by_h.pop(h)

        # ---------------- region 1: projections + first CAP units ----------
        n_emitted = 0
        with tc.tile_pool(name="phA", bufs=1) as phA, \
             tc.tile_pool(name="wst", bufs=2) as wst, \
             tc.tile_pool(name="pp", bufs=4, space="PSUM") as pp:
            def load_w(dram):
                tiles = []
                for t in range(6):
                    wt = wst.tile([128, D], bf, tag=f"w{t}", name=f"w{t}")
                    nc.sync.dma_start(out=wt[:, :], in_=dram[t * 128:(t + 1) * 128, :])
                    tiles.append(wt)
                return tiles

            wk = load_w(wk_d)
            hsT = []
            for t in range(6):
                hst = phA.tile([128, SPAD], bf, tag=f"hsT{t}", name=f"hsT{t}")
                nc.sync.dma_start(out=hst[:, 0:512], in_=hsT_d[t * 128:(t + 1) * 128, 0:512])
                nc.vector.memset(hst[:, S:SPAD], 0.0)
                hsT.append(hst)
            for (h0, h1) in ((512, 1024), (1024, S)):
                for t in range(6):
                    nc.sync.dma_start(out=hsT[t][:, h0:h1], in_=hsT_d[t * 128:(t + 1) * 128, h0:h1])

            vst4 = vst[:, 0:NPAIR * NH * 65].rearrange("a (p h e) -> a p h e", p=NPAIR, h=NH)
            nc.vector.memset(vst4[:, :, :, 64:65], 1.0)
            vcls_sb = consts.tile([1, D], bf, tag="vcls", name="vcls")
            wv = [None]

            def emit_v_pair(p):
                # token-major V projection for one 98-token pair; psum chunks
                # reuse the "pq" tag so V interleaves with the q/k projection
                c0 = 1 + 98 * p
                for ci, (h0, hw) in enumerate(((0, 512), (512, 256))):
                    ps = pp.tile([128, 512], f32, tag="pq", name="pq")
                    for kt in range(6):
                        nc.tensor.matmul(
                            ps[:, :hw],
                            lhsT=hsT[kt][:, c0:c0 + 128],
                            rhs=wv[0][kt][:, h0:h0 + hw],
                            start=(kt == 0), stop=(kt == 5),
                        )
                    nh0 = h0 // 64
                    dst = vst4[:, p, nh0:nh0 + hw // 64, 0:64]
                    src = ps[0:98, :hw].rearrange("a (h e) -> a h e", e=64)
                    bvs = bv_bc[0:98, h0:h0 + hw].rearrange("a (h e) -> a h e", e=64)
                    nc.vector.tensor_add(dst, src, bvs)

            def emit_v_cls():
                # cls-token V row: [1, 768] out, +bv folded into the copy
                for (h0, hw) in ((0, 512), (512, 256)):
                    ps = pp.tile([128, 512], f32, tag="pq", name="pq")
                    for kt in range(6):
                        nc.tensor.matmul(
                            ps[0:1, :hw],
                            lhsT=hsT[kt][:, 0:1],
                            rhs=wv[0][kt][:, h0:h0 + hw],
                            start=(kt == 0), stop=(kt == 5),
                        )
                    nc.vector.tensor_add(vcls_sb[:, h0:h0 + hw], ps[0:1, :hw],
                                         bv_sb[0:1, h0:h0 + hw])

            # kT projection first (cls-score row needs the full kT col 0)
            for dt in range(6):
                for (c0, cw) in s_chunks:
                    ps = pp.tile([128, 512], f32, tag="pq", name="pq")
                    for kt in range(6):
                        nc.tensor.matmul(
                            ps[:, :cw],
                            lhsT=wk[kt][:, dt * 128:(dt + 1) * 128],
                            rhs=hsT[kt][:, c0:c0 + cw],
                            start=(kt == 0), stop=(kt == 5),
                        )
                    nc.scalar.copy(kT[dt][:, c0:c0 + cw], ps[:, :cw])
                # block-diag cls-key columns for the dense cls-key score row
                for half in range(2):
                    r0 = half * 64
                    nc.vector.tensor_copy(
                        bdk[r0:r0 + 64, dt, 2 * dt + half:2 * dt + half + 1],
                        kT[dt][r0:r0 + 64, 0:1],
                    )

            # qT projection (bias+scale folded into the PSUM->SBUF copy),
            # interleaved with per-head-pair score units and V pairs
            wq = load_w(wq_d)
            wv[0] = load_w(wv_d)
            vp = 0
            clsraw = phA.tile([NH, S], bf, tag="clsraw", name="clsraw")
            for dt in range(6):
                for (c0, cw) in s_chunks:
                    ps = pp.tile([128, 512], f32, tag="pq", name="pq")
                    for kt in range(6):
                        nc.tensor.matmul(
                            ps[:, :cw],
                            lhsT=wq[kt][:, dt * 128:(dt + 1) * 128],
                            rhs=hsT[kt][:, c0:c0 + cw],
                            start=(kt == 0), stop=(kt == 5),
                        )
                    nc.scalar.activation(qT[dt][:, c0:c0 + cw], ps[:, :cw], Ident,
                                         bias=bqp_sb[:, dt:dt + 1], scale=SCALE)
                for h in (2 * dt, 2 * dt + 1):
                    for g in range(ng):
                        if n_emitted < CAP:
                            emit_unit(h, g)
                            n_emitted += 1
                while vp < min(NPAIR, 2 + (dt + 1) * 2):
                    emit_v_pair(vp)
                    vp += 1
                    if vp == 1:
                        emit_v_cls()
                if dt == 5:
                    # dense cls-key score row: [12, S] in 512-col psum chunks
                    # (borrows the score-tile psum pool)
                    for (c0, cw) in s_chunks:
                        cpt = scps.tile([128, 1024], f32, tag="sc", name="sc")
                        cp = cpt[0:12, 0:512]
                        for t in range(6):
                            nc.tensor.matmul(
                                cp[:, :cw],
                                lhsT=bdk[:, t, :],
                                rhs=qT[t][:, c0:c0 + cw],
                                start=(t == 0), stop=(t == 5),
                            )
                        nc.scalar.activation(clsraw[:, c0:c0 + cw], cp[:, :cw], Exp)
                        nc.vector.tensor_mul(atc[:, c0:c0 + cw], clsraw[:, c0:c0 + cw],
                                             ebc_sb[:, c0:c0 + cw])
            while vp < NPAIR:
                emit_v_pair(vp)
                vp += 1
            for h in range(NH):
                nc.sync.dma_start(
                    out=bdv[h:h + 1, h * 65:h * 65 + 64],
                    in_=vcls_sb[0:1, h * 64:(h + 1) * 64],
                )

        # ---------------- region 2: remaining units + AV pipeline ----------
        with tc.tile_pool(name="otps", bufs=1, space="PSUM") as otps_, \
             tc.tile_pool(name="obp", bufs=3) as obp_, \
             tc.tile_pool(name="nrm", bufs=2) as nrm_, \
             tc.tile_pool(name="drp", bufs=2, space="DRAM") as drp_:
            otps, obp, nrm, drp = otps_, obp_, nrm_, drp_
            rem = units[n_emitted:]
            for i, (h, g) in enumerate(rem):
                if pending:
                    emit_av(*pending.popleft())
                if pending and (i < 32 or len(rem) - i <= len(pending)):
                    emit_av(*pending.popleft())
                emit_unit(h, g)
            while pending:
                emit_av(*pending.popleft())

    _split_excess_waits(nc, mybir, limit=1)
    return nc


def _bench_pjrt(nc, in_maps, n_cores, iters=20, warmup=3):
    """Time repeated executions of the compiled kernel (no donation; inputs
    stay device-resident).  Returns (per_iter_ns, results_list)."""
    import time

    import jax
    import numpy as np
    from jax.sharding import Mesh, PartitionSpec
    from jax.experimental.shard_map import shard_map

    from concourse import mybir
    from concourse.bass2jax import (_bass_exec_p, install_neuronx_cc_hook,
                                    partition_id_tensor)

    install_neuronx_cc_hook()
    partition_name = nc.partition_id_tensor.name if nc.partition_id_tensor else None
    in_names, out_names, out_avals, zero_outs = [], [], [], []
    for alloc in nc.m.functions[0].allocations:
        if not isinstance(alloc, mybir.MemoryLocationSet):
            continue
        name = alloc.memorylocations[0].name
        if alloc.kind == "ExternalInput":
            if name != partition_name:
                in_names.append(name)
        elif alloc.kind == "ExternalOutput":
            shape = tuple(alloc.tensor_shape)
            dtype = mybir.dt.np(alloc.dtype)
            out_names.append(name)
            out_avals.append(jax.core.ShapedArray(shape, dtype))
            zero_outs.append(np.zeros(shape, dtype))
    n_params = len(in_names)
    all_in_names = in_names + out_names + ([partition_name] if partition_name else [])

    def _body(*args):
        operands = list(args)
        if partition_name is not None:
            operands.append(partition_id_tensor())
        return tuple(_bass_exec_p.bind(
            *operands,
            out_avals=tuple(out_avals),
            in_names=tuple(all_in_names),
            out_names=tuple(out_names),
            lowering_input_output_aliases=(),
            sim_require_finite=True,
            sim_require_nnan=True,
            nc=nc,
        ))

    devices = jax.devices()[:n_cores]
    mesh = Mesh(np.asarray(devices), ("core",))
    n_outs = len(out_names)
    sharded = jax.jit(
        shard_map(_body, mesh=mesh,
                  in_specs=(PartitionSpec("core"),) * (n_params + n_outs),
                  out_specs=(PartitionSpec("core"),) * n_outs,
                  check_rep=False),
        keep_unused=True,
    )
    per_core = [[np.asarray(m[name]) for name in in_names] for m in in_maps]
    concat_in = [np.concatenate([per_core[c][i] for c in range(n_cores)], axis=0)
                 for i in range(n_params)]
    concat_zeros = [np.zeros((n_cores * z.shape[0], *z.shape[1:]), z.dtype)
                    for z in zero_outs]
    dev_in = [jax.device_put(a) for a in concat_in + concat_zeros]
    out = sharded(*dev_in)
    jax.block_until_ready(out)
    for _ in range(warmup):
        out = sharded(*dev_in)
    jax.block_until_ready(out)
    t0 = time.perf_counter()
    for _ in range(iters):
        out = sharded(*dev_in)
    jax.block_until_ready(out)
    dt = (time.perf_counter() - t0) / iters
    results = [
        {name: np.asarray(out[i]).reshape(n_cores, *out_avals[i].shape)[c]
         for i, name in enumerate(out_names)}
        for c in range(n_cores)
    ]
    return int(dt * 1e9), results


# ----------------------------------------------------------------------------
# public entry point
# ----------------------------------------------------------------------------

def kernel(hidden_states, Wq, bq, Wk, Wv, bv, rel_table, rel_pos_index, rand_idx):
    import ml_dtypes

    import concourse.bass as bass
    import concourse.tile as tile
    from concourse import mybir
    from concourse.bass_utils import run_bass_kernel_spmd

    _patch_tile_drain()
    bf16 = ml_dtypes.bfloat16

    hidden_states = np.asarray(hidden_states, np.float32)
    Wq = np.asarray(Wq, np.float32)
    Wk = np.asarray(Wk, np.float32)
    Wv = np.asarray(Wv, np.float32)
    bq = np.asarray(bq, np.float32)
    bv = np.asarray(bv, np.float32)
    rel_table = np.asarray(rel_table, np.float32)
    rel_pos_index = np.asarray(rel_pos_index)
    rand_idx = np.asarray(rand_idx)

    lay = _build_layout(rand_idx)
    eb = _build_ebias(lay, rel_table, rel_pos_index).astype(bf16)
    ebc = _build_ebias_cls(rel_table, rel_pos_index).astype(bf16)
    bdo = np.zeros((NH, NH * 65 + 64), np.float32)
    for h in range(NH):
        bdo[h, h * 65 + 64] = 1.0
    bdo = bdo.astype(bf16)
    bqp = (bq.reshape(6, 128).T * SCALE).astype(np.float32).copy()

    shared = {
        "Wq": Wq.astype(bf16), "Wk": Wk.astype(bf16), "Wv": Wv.astype(bf16),
        "bqp": bqp,
        "bv_row": bv.reshape(1, D).astype(bf16),
        "ebias": eb, "ebias_cls": ebc, "bd_ones": bdo,
    }
    in_maps = []
    for b in range(B):
        m = dict(shared)
        m["hsT"] = np.ascontiguousarray(hidden_states[b].T).astype(bf16)
        in_maps.append(m)

    nc = bass.Bass()
    _emit(nc, tile, mybir, lay)

    kernel.last_nc = nc
    kernel.last_in_maps = in_maps
    bench_iters = int(os.environ.get("BEIT_BENCH", "0"))
    if bench_iters > 0:
        per_iter_ns, results = _bench_pjrt(nc, in_maps, N_CORES, iters=bench_iters)
        kernel.last_exec_time_ns = per_iter_ns
    else:
        res = run_bass_kernel_spmd(nc, in_maps, core_ids=list(range(N_CORES)))
        results = res.results

    out = np.empty((B, S, NH * DH), np.float32)
    for b in range(B):
        o = results[b]["out_t"].astype(np.float32)  # [NH, DH, S]
        out[b] = o.transpose(2, 0, 1).reshape(S, NH * DH)
    return out
